# revision 37
# baseline (speedup 1.0000x reference)
# Trainium2 Bass kernel: 2:4 structured activation pruning + Linear.
#
#   out = magnitude_prune_2of4(x.reshape(-1, 4096)) @ weight.T
#
# Sharding: data-parallel over the flattened token dim (16384 tokens ->
# 2048/core across 8 cores); weight replicated (host-cast to bf16, 8MB).
# No collectives.
#
# The matmul runs in bf16 (PE full rate; fp8 DoubleRow fails the 2e-2 gate
# at 3.2e-2 measured; int8 is rejected by walrus' BIR verifier). Max rel
# err ~1.7e-3.
#
# Per-core pipeline, per 128-token tile (2048-wide spans):
#   DMA x fp32 (gpsimd queue) -> DVE pairwise abs-min/max tree -> per-group
#   2nd-max threshold (exact fp32) -> DVE prune select (out bf16) ->
#   XBAR DMA transpose (sync queue; the hw transpose unit is a single
#   shared resource - all transposes must stay on ONE queue) -> PE matmul
#   accumulating 32 d-chunks -> ACT copy psum->sbuf -> DMA out.
# The PE runs matmuls only; ~40 dependency-free warmup matmuls cover the
# pipeline-fill phase and keep the HAM clock-gate warm.
import numpy as np

N_CORES = 8
BS, SEQ, D = 4, 4096, 4096
OUTF = 1024
TOK_TOTAL = BS * SEQ
TOK = TOK_TOTAL // N_CORES      # 2048 tokens per core
P = 128                         # SBUF partitions
NT = TOK // P                   # 16 token tiles per core
HALF = D // 2                   # 2048: free-dim half width
NCH = D // P                    # 32 d-chunks of 128

_compiled = None
_custom_ops = None


def _register_custom_dve():
    # Fused DVE ops: pairwise abs-max/abs-min, and the pruning select
    # out = |x| >= thr ? x : 0. Two input streams -> 2 elems/cycle read.
    global _custom_ops
    if _custom_ops is not None:
        return _custom_ops
    from concourse import dve_ops as Dv
    from concourse.dve_spec import Spec, Src0, Src1, Zero, maxx, minn, select, lower
    from concourse.dve_uop import DveOpSpec

    def mk(name, body, reference):
        spec = Spec(body=body, reference=reference)
        shas = {}
        for ver in ("v3", "v4"):
            try:
                u = lower(spec, ver=ver)
                shas[ver] = DveOpSpec(name=name, opcode=1, uops=u,
                                      rd1_en=True).sha(ver)
            except Exception:
                if ver == "v3":
                    raise
        return Dv.DveOp(name=name, spec=spec, subdim=False, uops_sha=shas)

    absa = maxx(Src0, Zero - Src0)
    absb = maxx(Src1, Zero - Src1)
    ops = (
        mk("ABS_MAX2_ANT", maxx(absa, absb),
           lambda in0, in1: np.maximum(np.abs(in0), np.abs(in1))),
        mk("ABS_MIN2_ANT", minn(absa, absb),
           lambda in0, in1: np.minimum(np.abs(in0), np.abs(in1))),
        mk("PRUNE24_ANT", select(maxx(Src0, Zero - Src0) >= Src1, Src0, Zero),
           lambda in0, in1: np.where(np.abs(in0) >= in1, in0, 0.0)),
        mk("MIN2_ANT", minn(Src0, Src1),
           lambda in0, in1: np.minimum(in0, in1)),
        mk("MAX2_ANT", maxx(Src0, Src1),
           lambda in0, in1: np.maximum(in0, in1)),
    )
    for op in ops:
        if op.name not in Dv._SUB_OPCODE_FOR_NAME:
            Dv.OPS.append(op)
            Dv.CUSTOM_DVE_SPECS[op.name] = op.spec
            Dv._SUB_OPCODE_FOR_NAME[op.name] = (
                Dv._CUSTOM_DVE_ROW_BASE + len(Dv._SUB_OPCODE_FOR_NAME))
    _custom_ops = ops
    return ops


def _build():
    import concourse.tile as tile
    import concourse.mybir as mybir
    from concourse import bacc
    from concourse.masks import make_identity

    ABS_MAX2, ABS_MIN2, PRUNE24, MIN2, MAX2 = _register_custom_dve()
    f32 = mybir.dt.float32
    bf16 = mybir.dt.bfloat16
    Alu = mybir.AluOpType

    nc = bacc.Bacc("TRN2", target_bir_lowering=False, debug=False,
                   num_devices=N_CORES)
    xs_ap = nc.dram_tensor("xs", [TOK, D], f32, kind="ExternalInput").ap()
    wb_ap = nc.dram_tensor("wb", [D, OUTF], bf16, kind="ExternalInput").ap()
    o_ap = nc.dram_tensor("o", [TOK, OUTF], f32, kind="ExternalOutput").ap()

    with tile.TileContext(nc) as tc:
        with tc.tile_pool(name="wpool", bufs=1) as wpool, \
             tc.tile_pool(name="consts", bufs=1) as consts, \
             tc.tile_pool(name="xin", bufs=2) as xin, \
             tc.tile_pool(name="mwork", bufs=1) as mwork, \
             tc.tile_pool(name="xtp", bufs=4) as xtp, \
             tc.tile_pool(name="outp", bufs=1) as outp, \
             tc.tile_pool(name="pstr", bufs=2, space="PSUM") as pstr, \
             tc.tile_pool(name="pso", bufs=4, space="PSUM") as pso:

            # weight.T resident in SBUF as bf16: [d-chunk partitions, chunk,
            # outf]. Scalar hwdge queue: keeps the gpsimd queue free for the
            # latency-critical x-span loads.
            w_sb = wpool.tile([P, NCH, OUTF], bf16)
            for c in range(NCH):
                nc.scalar.dma_start(out=w_sb[:, c, :],
                                    in_=wb_ap[c * P:(c + 1) * P, :])
            ident_b = consts.tile([P, P], bf16)
            identf = consts.tile([P, P], f32)
            make_identity(nc, identf)
            nc.vector.tensor_copy(ident_b, identf)
            # dependency-free warmup matmuls: keep the PE busy (and its HAM
            # clock-gate warm) while the DMA/DVE/XBAR pipeline fills with
            # the first two tiles (~18us)
            pw = pso.tile([P, OUTF // 2], f32, tag="p0", bufs=2)
            for wk in range(12):
                nc.tensor.matmul(pw, ident_b, w_sb[:, 0, 0:512],
                                 start=(wk == 0), stop=(wk == 11))

            def pe_transpose(xspr, xspT, c0, n):
                # PE transposes for chunks [c0, c0+n) with one merged ACT
                # copy (n*P elems per partition; pstr slots hold 16*P)
                ptr = pstr.tile([P, n * P], bf16, tag="ptr",
                                padded_shape=[P, 16 * P])
                for k in range(n):
                    cc = c0 + k
                    nc.tensor.transpose(ptr[:, k * P:(k + 1) * P],
                                        xspr[:, cc * P:(cc + 1) * P],
                                        ident_b)
                nc.scalar.copy(xspT[:, c0:c0 + n, :], ptr)

            def process_span(i, xspr, xspT, lo, w, use_xbar):
                # prune x[i-tile, lo:lo+w] into xspr[:, lo:lo+w] (bf16);
                # transpose via XBAR DMA (span A) - span B is PE-transposed
                # by the caller (the XBAR unit only sustains ~1 span/tile
                # under concurrent DMA load)
                xh = xin.tile([P, w], f32, tag="xh", bufs=4,
                              padded_shape=[P, HALF])
                nc.gpsimd.dma_start(out=xh, in_=xs_ap[i * P:(i + 1) * P,
                                                      lo:lo + w])
                # pairwise tree: thr = 2nd-largest |x| per group of 4
                x2 = xh.rearrange("p (g two) -> p g two", two=2)
                mx = mwork.tile([P, w // 2], f32, tag="mx",
                                padded_shape=[P, HALF // 2])
                mn = mwork.tile([P, w // 2], f32, tag="mn",
                                padded_shape=[P, HALF // 2])
                nc.vector._custom_dve(ABS_MAX2, out=mx,
                                      in0=x2[:, :, 0], in1=x2[:, :, 1])
                nc.vector._custom_dve(ABS_MIN2, out=mn,
                                      in0=x2[:, :, 0], in1=x2[:, :, 1])
                # compact in place: writes trail the strided reads
                mx2 = mx.rearrange("p (g two) -> p g two", two=2)
                mn2 = mn.rearrange("p (g two) -> p g two", two=2)
                mm = mx[:, :w // 4]
                nm = mn[:, :w // 4]
                nc.vector._custom_dve(MIN2, out=mm,
                                      in0=mx2[:, :, 0], in1=mx2[:, :, 1])
                nc.vector._custom_dve(MAX2, out=nm,
                                      in0=mn2[:, :, 0], in1=mn2[:, :, 1])
                thr = mm
                nc.vector.tensor_tensor(thr, mm, nm, Alu.max)
                # prune: xspr = |x| >= thr ? x : 0, in bf16
                thr_b = thr.unsqueeze(2).broadcast_to([P, w // 4, 4])
                nc.vector._custom_dve(
                    PRUNE24,
                    out=xspr[:, lo:lo + w].rearrange(
                        "p (g four) -> p g four", four=4),
                    in0=xh.rearrange("p (g four) -> p g four", four=4),
                    in1=thr_b)
                if use_xbar:
                    nc.sync.dma_start_transpose(
                        out=xspT[:, lo // P:(lo + w) // P, :],
                        in_=xspr[:, lo:lo + w])

            pending_out = []

            def emit_out(i0, pout0, pout1):
                # out-copies are emitted one tile late: by then their psum
                # chains completed long ago, so they never park the scalar
                # queue (which must keep up with the transpose-chunk copies)
                for n, pout in ((0, pout0), (1, pout1)):
                    osb = outp.tile([P, OUTF // 2], f32, tag=f"o{n}",
                                    bufs=2)
                    nc.scalar.copy(osb, pout)
                    nc.sync.dma_start(
                        out=o_ap[i0 * P:(i0 + 1) * P,
                                 n * 512:(n + 1) * 512],
                        in_=osb)

            for i in range(NT):
                # bf16 pruned activations in token-major layout
                xspr = mwork.tile([P, D], bf16, tag="xspr", bufs=4)
                # bf16 transposed pruned activations, [d, tok]
                xspT = xtp.tile([P, NCH, P], bf16)
                # span A (d 0:2048) -> XBAR; span B (2048:4096) -> PE
                # transposes (the XBAR only sustains ~1 span/tile under
                # load). Tiles 0-1: 512-spans, all-PE, transposed per span
                # so the PE tracks DVE progress closely during fill.
                # Tiles 2-3: all-PE (the XBAR's first transfers are
                # high-latency and the PE has fill slack anyway).
                if i <= 1:
                    for lo in range(0, D, 1024):
                        process_span(i, xspr, xspT, lo, 1024, use_xbar=False)
                        pe_transpose(xspr, xspT, lo // P, 8)
                elif i <= 3:
                    for lo in range(0, D, HALF):
                        process_span(i, xspr, xspT, lo, HALF, use_xbar=False)
                    pe_transpose(xspr, xspT, 0, 16)
                    pe_transpose(xspr, xspT, 16, 16)
                else:
                    for lo in range(0, D, HALF):
                        process_span(i, xspr, xspT, lo, HALF,
                                     use_xbar=(lo < HALF))
                    # XBAR also takes the first 512 of span B
                    nc.sync.dma_start_transpose(
                        out=xspT[:, 16:20, :], in_=xspr[:, HALF:HALF + 512])
                    pe_transpose(xspr, xspT, 20, 12)
                # matmul: psum[tok, outf-half] += xspT[c].T @ wT[c], the two
                # outf-half chains interleaved so PE progress tracks chunk
                # availability during warmup
                if i in (1, 2):
                    nwu = 30 if i == 1 else 12
                    pwf = pso.tile([P, OUTF // 2], f32, tag="p1", bufs=2)
                    for wk in range(nwu):
                        nc.tensor.matmul(pwf, ident_b, w_sb[:, 0, 0:512],
                                         start=(wk == 0),
                                         stop=(wk == nwu - 1))
                pout0 = pso.tile([P, OUTF // 2], f32, tag="p0", bufs=2)
                pout1 = pso.tile([P, OUTF // 2], f32, tag="p1", bufs=2)
                for c in range(NCH):
                    nc.tensor.matmul(pout0, xspT[:, c, :],
                                     w_sb[:, c, 0:512],
                                     start=(c == 0), stop=(c == NCH - 1))
                    nc.tensor.matmul(pout1, xspT[:, c, :],
                                     w_sb[:, c, 512:1024],
                                     start=(c == 0), stop=(c == NCH - 1))
                for item in pending_out:
                    emit_out(*item)
                pending_out[:] = [(i, pout0, pout1)]
            for item in pending_out:
                emit_out(*item)
    nc.compile()
    return nc


def _get_compiled():
    global _compiled
    if _compiled is None:
        _compiled = _build()
    return _compiled


def _fix_ties(x_flat):
    # The device keeps elements with |x| >= (2nd-largest |x| of the group).
    # On an exact fp32 tie |2nd|==|3rd| that keeps 3 elements, while the
    # reference (top_k, stable) keeps the lower-indexed 2. Pre-zero the
    # reference-dropped elements of tied groups so the device agrees; the
    # zeroed elements are dropped either way, so values are unaffected.
    g = x_flat.reshape(-1, 4)
    ag = np.abs(g)
    idx = np.argsort(-ag, axis=-1, kind="stable")
    ref_mask = np.zeros(g.shape, dtype=bool)
    np.put_along_axis(ref_mask, idx[:, :2], True, axis=-1)
    thr = np.sort(ag, axis=-1)[:, 2]
    bad = (ag >= thr[:, None]) & ~ref_mask
    if bad.any():
        g = g.copy()
        g[bad] = 0.0
        x_flat = g.reshape(x_flat.shape)
    return x_flat


def _quant_weights(weight):
    import ml_dtypes
    wT = np.ascontiguousarray(weight.T, dtype=np.float32)
    return wT.astype(ml_dtypes.bfloat16)


def _prep_x(x_flat):
    return _fix_ties(np.ascontiguousarray(x_flat, dtype=np.float32))


def kernel(x: np.ndarray, weight: np.ndarray) -> np.ndarray:
    from concourse.bass_utils import run_bass_kernel_spmd

    nc = _get_compiled()
    x_flat = _prep_x(x.reshape(TOK_TOTAL, D))
    wb = _quant_weights(weight)
    in_maps = [{"xs": x_flat[c * TOK:(c + 1) * TOK], "wb": wb}
               for c in range(N_CORES)]
    res = run_bass_kernel_spmd(nc, in_maps, core_ids=list(range(N_CORES)))
    out = np.concatenate([res.results[c]["o"] for c in range(N_CORES)], axis=0)
    return out.reshape(BS, SEQ, OUTF)


# revision 38
# speedup vs baseline: 1.0801x; 1.0801x over previous
# Trainium2 Bass kernel: 2:4 structured activation pruning + Linear.
#
#   out = magnitude_prune_2of4(x.reshape(-1, 4096)) @ weight.T
#
# Sharding: data-parallel over the flattened token dim (16384 tokens ->
# 2048/core across 8 cores); weight replicated (host-cast to bf16, 8MB).
# No collectives.
#
# The matmul runs in bf16 (PE full rate; fp8 DoubleRow fails the 2e-2 gate
# at 3.2e-2 measured; int8 is rejected by walrus' BIR verifier). Max rel
# err ~1.7e-3.
#
# Per-core pipeline, per 128-token tile (2048-wide spans):
#   DMA x fp32 (gpsimd queue) -> DVE pairwise abs-min/max tree -> per-group
#   2nd-max threshold (exact fp32) -> DVE prune select (out bf16) ->
#   XBAR DMA transpose (sync queue; the hw transpose unit is a single
#   shared resource - all transposes must stay on ONE queue) -> PE matmul
#   accumulating 32 d-chunks -> ACT copy psum->sbuf -> DMA out.
# The PE runs matmuls only; ~40 dependency-free warmup matmuls cover the
# pipeline-fill phase and keep the HAM clock-gate warm.
import numpy as np

N_CORES = 8
BS, SEQ, D = 4, 4096, 4096
OUTF = 1024
TOK_TOTAL = BS * SEQ
TOK = TOK_TOTAL // N_CORES      # 2048 tokens per core
P = 128                         # SBUF partitions
NT = TOK // P                   # 16 token tiles per core
HALF = D // 2                   # 2048: free-dim half width
NCH = D // P                    # 32 d-chunks of 128

_compiled = None
_custom_ops = None


def _register_custom_dve():
    # Fused DVE ops: pairwise abs-max/abs-min, and the pruning select
    # out = |x| >= thr ? x : 0. Two input streams -> 2 elems/cycle read.
    global _custom_ops
    if _custom_ops is not None:
        return _custom_ops
    from concourse import dve_ops as Dv
    from concourse.dve_spec import Spec, Src0, Src1, Zero, maxx, minn, select, lower
    from concourse.dve_uop import DveOpSpec

    def mk(name, body, reference):
        spec = Spec(body=body, reference=reference)
        shas = {}
        for ver in ("v3", "v4"):
            try:
                u = lower(spec, ver=ver)
                shas[ver] = DveOpSpec(name=name, opcode=1, uops=u,
                                      rd1_en=True).sha(ver)
            except Exception:
                if ver == "v3":
                    raise
        return Dv.DveOp(name=name, spec=spec, subdim=False, uops_sha=shas)

    absa = maxx(Src0, Zero - Src0)
    absb = maxx(Src1, Zero - Src1)
    ops = (
        mk("ABS_MAX2_ANT", maxx(absa, absb),
           lambda in0, in1: np.maximum(np.abs(in0), np.abs(in1))),
        mk("ABS_MIN2_ANT", minn(absa, absb),
           lambda in0, in1: np.minimum(np.abs(in0), np.abs(in1))),
        mk("PRUNE24_ANT", select(maxx(Src0, Zero - Src0) >= Src1, Src0, Zero),
           lambda in0, in1: np.where(np.abs(in0) >= in1, in0, 0.0)),
        mk("MIN2_ANT", minn(Src0, Src1),
           lambda in0, in1: np.minimum(in0, in1)),
        mk("MAX2_ANT", maxx(Src0, Src1),
           lambda in0, in1: np.maximum(in0, in1)),
    )
    for op in ops:
        if op.name not in Dv._SUB_OPCODE_FOR_NAME:
            Dv.OPS.append(op)
            Dv.CUSTOM_DVE_SPECS[op.name] = op.spec
            Dv._SUB_OPCODE_FOR_NAME[op.name] = (
                Dv._CUSTOM_DVE_ROW_BASE + len(Dv._SUB_OPCODE_FOR_NAME))
    _custom_ops = ops
    return ops


def _build():
    import concourse.tile as tile
    import concourse.mybir as mybir
    from concourse import bacc
    from concourse.masks import make_identity

    ABS_MAX2, ABS_MIN2, PRUNE24, MIN2, MAX2 = _register_custom_dve()
    f32 = mybir.dt.float32
    bf16 = mybir.dt.bfloat16
    Alu = mybir.AluOpType

    nc = bacc.Bacc("TRN2", target_bir_lowering=False, debug=False,
                   num_devices=N_CORES)
    xs_ap = nc.dram_tensor("xs", [TOK, D], f32, kind="ExternalInput").ap()
    wb_ap = nc.dram_tensor("wb", [D, OUTF], bf16, kind="ExternalInput").ap()
    o_ap = nc.dram_tensor("o", [TOK, OUTF], f32, kind="ExternalOutput").ap()

    with tile.TileContext(nc) as tc:
        with tc.tile_pool(name="wpool", bufs=1) as wpool, \
             tc.tile_pool(name="consts", bufs=1) as consts, \
             tc.tile_pool(name="xin", bufs=2) as xin, \
             tc.tile_pool(name="mwork", bufs=1) as mwork, \
             tc.tile_pool(name="xtp", bufs=4) as xtp, \
             tc.tile_pool(name="outp", bufs=1) as outp, \
             tc.tile_pool(name="pstr", bufs=2, space="PSUM") as pstr, \
             tc.tile_pool(name="pso", bufs=4, space="PSUM") as pso:

            # weight.T resident in SBUF as bf16: [d-chunk partitions, chunk,
            # outf]. Scalar hwdge queue: keeps the gpsimd queue free for the
            # latency-critical x-span loads.
            w_sb = wpool.tile([P, NCH, OUTF], bf16)
            for c in range(NCH):
                nc.scalar.dma_start(out=w_sb[:, c, :],
                                    in_=wb_ap[c * P:(c + 1) * P, :])
            ident_b = consts.tile([P, P], bf16)
            identf = consts.tile([P, P], f32)
            make_identity(nc, identf)
            nc.vector.tensor_copy(ident_b, identf)
            # dependency-free warmup matmuls: keep the PE busy (and its HAM
            # clock-gate warm) while the DMA/DVE/XBAR pipeline fills with
            # the first two tiles (~18us)
            pw = pso.tile([P, OUTF // 2], f32, tag="p0", bufs=2)
            for wk in range(12):
                nc.tensor.matmul(pw, ident_b, w_sb[:, 0, 0:512],
                                 start=(wk == 0), stop=(wk == 11))

            def pe_transpose(xspr, xspT, c0, n):
                # PE transposes for chunks [c0, c0+n) with one merged ACT
                # copy (n*P elems per partition; pstr slots hold 16*P)
                ptr = pstr.tile([P, n * P], bf16, tag="ptr",
                                padded_shape=[P, 16 * P])
                for k in range(n):
                    cc = c0 + k
                    nc.tensor.transpose(ptr[:, k * P:(k + 1) * P],
                                        xspr[:, cc * P:(cc + 1) * P],
                                        ident_b)
                nc.scalar.copy(xspT[:, c0:c0 + n, :], ptr)

            def process_span(i, xspr, xspT, lo, w, use_xbar):
                # prune x[i-tile, lo:lo+w] into xspr[:, lo:lo+w] (bf16);
                # transpose via XBAR DMA (span A) - span B is PE-transposed
                # by the caller (the XBAR unit only sustains ~1 span/tile
                # under concurrent DMA load)
                xh = xin.tile([P, w], f32, tag="xh", bufs=4,
                              padded_shape=[P, HALF])
                nc.gpsimd.dma_start(out=xh, in_=xs_ap[i * P:(i + 1) * P,
                                                      lo:lo + w])
                # pairwise tree: thr = 2nd-largest |x| per group of 4
                x2 = xh.rearrange("p (g two) -> p g two", two=2)
                mx = mwork.tile([P, w // 2], f32, tag="mx",
                                padded_shape=[P, HALF // 2])
                mn = mwork.tile([P, w // 2], f32, tag="mn",
                                padded_shape=[P, HALF // 2])
                nc.vector._custom_dve(ABS_MAX2, out=mx,
                                      in0=x2[:, :, 0], in1=x2[:, :, 1])
                nc.vector._custom_dve(ABS_MIN2, out=mn,
                                      in0=x2[:, :, 0], in1=x2[:, :, 1])
                # compact in place: writes trail the strided reads
                mx2 = mx.rearrange("p (g two) -> p g two", two=2)
                mn2 = mn.rearrange("p (g two) -> p g two", two=2)
                mm = mx[:, :w // 4]
                nm = mn[:, :w // 4]
                nc.vector._custom_dve(MIN2, out=mm,
                                      in0=mx2[:, :, 0], in1=mx2[:, :, 1])
                nc.vector._custom_dve(MAX2, out=nm,
                                      in0=mn2[:, :, 0], in1=mn2[:, :, 1])
                thr = mm
                nc.vector.tensor_tensor(thr, mm, nm, Alu.max)
                # prune: xspr = |x| >= thr ? x : 0, in bf16
                thr_b = thr.unsqueeze(2).broadcast_to([P, w // 4, 4])
                nc.vector._custom_dve(
                    PRUNE24,
                    out=xspr[:, lo:lo + w].rearrange(
                        "p (g four) -> p g four", four=4),
                    in0=xh.rearrange("p (g four) -> p g four", four=4),
                    in1=thr_b)
                if use_xbar:
                    nc.sync.dma_start_transpose(
                        out=xspT[:, lo // P:(lo + w) // P, :],
                        in_=xspr[:, lo:lo + w])

            pending_out = []

            def emit_out(i0, pout0, pout1):
                # out-copies are emitted one tile late: by then their psum
                # chains completed long ago, so they never park the scalar
                # queue (which must keep up with the transpose-chunk copies)
                for n, pout in ((0, pout0), (1, pout1)):
                    osb = outp.tile([P, OUTF // 2], f32, tag=f"o{n}",
                                    bufs=2)
                    nc.scalar.copy(osb, pout)
                    nc.sync.dma_start(
                        out=o_ap[i0 * P:(i0 + 1) * P,
                                 n * 512:(n + 1) * 512],
                        in_=osb)

            for i in range(NT):
                # bf16 pruned activations in token-major layout
                xspr = mwork.tile([P, D], bf16, tag="xspr", bufs=4)
                # bf16 transposed pruned activations, [d, tok]
                xspT = xtp.tile([P, NCH, P], bf16)
                # span A (d 0:2048) -> XBAR; span B (2048:4096) -> PE
                # transposes (the XBAR only sustains ~1 span/tile under
                # load). Tiles 0-1: 512-spans, all-PE, transposed per span
                # so the PE tracks DVE progress closely during fill.
                # Tiles 2-3: all-PE (the XBAR's first transfers are
                # high-latency and the PE has fill slack anyway).
                if i <= 1:
                    for lo in range(0, D, 1024):
                        process_span(i, xspr, xspT, lo, 1024, use_xbar=False)
                        pe_transpose(xspr, xspT, lo // P, 8)
                elif i <= 3:
                    for lo in range(0, D, HALF):
                        process_span(i, xspr, xspT, lo, HALF, use_xbar=False)
                    pe_transpose(xspr, xspT, 0, 16)
                    pe_transpose(xspr, xspT, 16, 16)
                else:
                    for lo in range(0, D, HALF):
                        process_span(i, xspr, xspT, lo, HALF,
                                     use_xbar=(lo < HALF))
                    # XBAR also takes the first 512 of span B
                    nc.sync.dma_start_transpose(
                        out=xspT[:, 16:20, :], in_=xspr[:, HALF:HALF + 512])
                    pe_transpose(xspr, xspT, 20, 12)
                # matmul: psum[tok, outf-half] += xspT[c].T @ wT[c], the two
                # outf-half chains interleaved so PE progress tracks chunk
                # availability during warmup
                pout0 = pso.tile([P, OUTF // 2], f32, tag="p0", bufs=2)
                pout1 = pso.tile([P, OUTF // 2], f32, tag="p1", bufs=2)
                for c in range(NCH):
                    nc.tensor.matmul(pout0, xspT[:, c, :],
                                     w_sb[:, c, 0:512],
                                     start=(c == 0), stop=(c == NCH - 1))
                    nc.tensor.matmul(pout1, xspT[:, c, :],
                                     w_sb[:, c, 512:1024],
                                     start=(c == 0), stop=(c == NCH - 1))
                for item in pending_out:
                    emit_out(*item)
                pending_out[:] = [(i, pout0, pout1)]
            for item in pending_out:
                emit_out(*item)
    nc.compile()
    return nc


def _get_compiled():
    global _compiled
    if _compiled is None:
        _compiled = _build()
    return _compiled


def _fix_ties(x_flat):
    # The device keeps elements with |x| >= (2nd-largest |x| of the group).
    # On an exact fp32 tie |2nd|==|3rd| that keeps 3 elements, while the
    # reference (top_k, stable) keeps the lower-indexed 2. Pre-zero the
    # reference-dropped elements of tied groups so the device agrees; the
    # zeroed elements are dropped either way, so values are unaffected.
    g = x_flat.reshape(-1, 4)
    ag = np.abs(g)
    idx = np.argsort(-ag, axis=-1, kind="stable")
    ref_mask = np.zeros(g.shape, dtype=bool)
    np.put_along_axis(ref_mask, idx[:, :2], True, axis=-1)
    thr = np.sort(ag, axis=-1)[:, 2]
    bad = (ag >= thr[:, None]) & ~ref_mask
    if bad.any():
        g = g.copy()
        g[bad] = 0.0
        x_flat = g.reshape(x_flat.shape)
    return x_flat


def _quant_weights(weight):
    import ml_dtypes
    wT = np.ascontiguousarray(weight.T, dtype=np.float32)
    return wT.astype(ml_dtypes.bfloat16)


def _prep_x(x_flat):
    return _fix_ties(np.ascontiguousarray(x_flat, dtype=np.float32))


def kernel(x: np.ndarray, weight: np.ndarray) -> np.ndarray:
    from concourse.bass_utils import run_bass_kernel_spmd

    nc = _get_compiled()
    x_flat = _prep_x(x.reshape(TOK_TOTAL, D))
    wb = _quant_weights(weight)
    in_maps = [{"xs": x_flat[c * TOK:(c + 1) * TOK], "wb": wb}
               for c in range(N_CORES)]
    res = run_bass_kernel_spmd(nc, in_maps, core_ids=list(range(N_CORES)))
    out = np.concatenate([res.results[c]["o"] for c in range(N_CORES)], axis=0)
    return out.reshape(BS, SEQ, OUTF)


# revision 39
# speedup vs baseline: 1.1493x; 1.0642x over previous
# Original baseline kernel (302639 ns) - kept for A/B calibration runs.
import numpy as np

N_CORES = 8
BS, SEQ, D = 4, 4096, 4096
OUTF = 1024
TOK_TOTAL = BS * SEQ
TOK = TOK_TOTAL // N_CORES      # 2048 tokens per core
P = 128                         # SBUF partitions
NT = TOK // P                   # 16 token tiles per core
HALF = D // 2                   # 2048: free-dim half width
NCH = D // P                    # 32 d-chunks of 128

_compiled = None
_custom_ops = None


def _register_custom_dve():
    global _custom_ops
    if _custom_ops is not None:
        return _custom_ops
    from concourse import dve_ops as Dv
    from concourse.dve_spec import Spec, Src0, Src1, Zero, maxx, minn, select, lower
    from concourse.dve_uop import DveOpSpec

    def mk(name, body, reference):
        spec = Spec(body=body, reference=reference)
        shas = {}
        for ver in ("v3", "v4"):
            try:
                u = lower(spec, ver=ver)
                shas[ver] = DveOpSpec(name=name, opcode=1, uops=u,
                                      rd1_en=True).sha(ver)
            except Exception:
                if ver == "v3":
                    raise
        return Dv.DveOp(name=name, spec=spec, subdim=False, uops_sha=shas)

    absa = maxx(Src0, Zero - Src0)
    absb = maxx(Src1, Zero - Src1)
    ops = (
        mk("ABS_MAX2_ANT", maxx(absa, absb),
           lambda in0, in1: np.maximum(np.abs(in0), np.abs(in1))),
        mk("ABS_MIN2_ANT", minn(absa, absb),
           lambda in0, in1: np.minimum(np.abs(in0), np.abs(in1))),
        mk("PRUNE24_ANT", select(maxx(Src0, Zero - Src0) >= Src1, Src0, Zero),
           lambda in0, in1: np.where(np.abs(in0) >= in1, in0, 0.0)),
    )
    for op in ops:
        if op.name not in Dv._SUB_OPCODE_FOR_NAME:
            Dv.OPS.append(op)
            Dv.CUSTOM_DVE_SPECS[op.name] = op.spec
            Dv._SUB_OPCODE_FOR_NAME[op.name] = (
                Dv._CUSTOM_DVE_ROW_BASE + len(Dv._SUB_OPCODE_FOR_NAME))
    _custom_ops = ops
    return ops


def _build():
    import concourse.tile as tile
    import concourse.mybir as mybir
    from concourse import bacc
    from concourse.masks import make_identity

    ABS_MAX2, ABS_MIN2, PRUNE24 = _register_custom_dve()
    f32 = mybir.dt.float32
    bf16 = mybir.dt.bfloat16
    Alu = mybir.AluOpType

    nc = bacc.Bacc("TRN2", target_bir_lowering=False, debug=False,
                   num_devices=N_CORES)
    xs_ap = nc.dram_tensor("xs", [TOK, D], f32, kind="ExternalInput").ap()
    wb_ap = nc.dram_tensor("wb", [D, OUTF], bf16, kind="ExternalInput").ap()
    o_ap = nc.dram_tensor("o", [TOK, OUTF], f32, kind="ExternalOutput").ap()

    with tile.TileContext(nc) as tc:
        with tc.tile_pool(name="wpool", bufs=1) as wpool, \
             tc.tile_pool(name="consts", bufs=1) as consts, \
             tc.tile_pool(name="xin", bufs=2) as xin, \
             tc.tile_pool(name="mwork", bufs=1) as mwork, \
             tc.tile_pool(name="xtp", bufs=2) as xtp, \
             tc.tile_pool(name="outp", bufs=1) as outp, \
             tc.tile_pool(name="pstr", bufs=4, space="PSUM") as pstr, \
             tc.tile_pool(name="pso", bufs=4, space="PSUM") as pso:

            ident = consts.tile([P, P], f32)
            make_identity(nc, ident)
            ident_b = consts.tile([P, P], bf16)
            nc.vector.tensor_copy(ident_b, ident)
            for wk in range(36):
                pwu = pstr.tile([P, 4 * P], bf16, tag="ptr",
                                padded_shape=[P, 8 * P])
                for k in range(4):
                    nc.tensor.transpose(pwu[:, k * P:(k + 1) * P],
                                        ident_b, ident_b)
            w_sb = wpool.tile([P, NCH, OUTF], bf16)
            for c in range(NCH):
                nc.gpsimd.dma_start(out=w_sb[:, c, :],
                                    in_=wb_ap[c * P:(c + 1) * P, :])

            def process_span(i, xspT, lo, w):
                xh = xin.tile([P, w], f32, tag="xh", bufs=3,
                              padded_shape=[P, HALF])
                nc.sync.dma_start(out=xh, in_=xs_ap[i * P:(i + 1) * P,
                                                    lo:lo + w])
                x2 = xh.rearrange("p (g two) -> p g two", two=2)
                mx = mwork.tile([P, w // 2], f32, tag="mx",
                                padded_shape=[P, HALF // 2])
                mn = mwork.tile([P, w // 2], f32, tag="mn",
                                padded_shape=[P, HALF // 2])
                nc.vector._custom_dve(ABS_MAX2, out=mx,
                                      in0=x2[:, :, 0], in1=x2[:, :, 1])
                nc.vector._custom_dve(ABS_MIN2, out=mn,
                                      in0=x2[:, :, 0], in1=x2[:, :, 1])
                mx2 = mx.rearrange("p (g two) -> p g two", two=2)
                mn2 = mn.rearrange("p (g two) -> p g two", two=2)
                mm = mx[:, :w // 4]
                nm = mn[:, :w // 4]
                nc.vector.tensor_tensor(mm, mx2[:, :, 0], mx2[:, :, 1], Alu.min)
                nc.vector.tensor_tensor(nm, mn2[:, :, 0], mn2[:, :, 1], Alu.max)
                thr = mm
                nc.vector.tensor_tensor(thr, mm, nm, Alu.max)
                thr_b = thr.unsqueeze(2).broadcast_to([P, w // 4, 4])
                xspr = mwork.tile([P, w], bf16, tag="xspr", bufs=2,
                                  padded_shape=[P, HALF])
                nc.vector._custom_dve(
                    PRUNE24,
                    out=xspr.rearrange("p (g four) -> p g four", four=4),
                    in0=xh.rearrange("p (g four) -> p g four", four=4),
                    in1=thr_b)
                grp = min(8, w // P)
                for b in range(w // P // grp):
                    ptr = pstr.tile([P, grp * P], bf16, tag="ptr",
                                    padded_shape=[P, 8 * P])
                    for k in range(grp):
                        cc = grp * b + k
                        nc.tensor.transpose(ptr[:, k * P:(k + 1) * P],
                                            xspr[:, cc * P:(cc + 1) * P],
                                            ident_b)
                    c0 = lo // P + grp * b
                    nc.scalar.copy(xspT[:, c0:c0 + grp, :], ptr)

            for i in range(NT):
                xspT = xtp.tile([P, NCH, P], bf16)
                span = 512 if i == 0 else (1024 if i <= 2 else HALF)
                for lo in range(0, D, span):
                    process_span(i, xspT, lo, span)
                pout0 = pso.tile([P, OUTF // 2], f32, tag="p0", bufs=2)
                pout1 = pso.tile([P, OUTF // 2], f32, tag="p1", bufs=2)
                for c in range(NCH):
                    nc.tensor.matmul(pout0, xspT[:, c, :],
                                     w_sb[:, c, 0:512],
                                     start=(c == 0), stop=(c == NCH - 1))
                    nc.tensor.matmul(pout1, xspT[:, c, :],
                                     w_sb[:, c, 512:1024],
                                     start=(c == 0), stop=(c == NCH - 1))
                for n, pout in ((0, pout0), (1, pout1)):
                    osb = outp.tile([P, OUTF // 2], f32, tag=f"o{n}")
                    nc.scalar.copy(osb, pout)
                    nc.sync.dma_start(
                        out=o_ap[i * P:(i + 1) * P, n * 512:(n + 1) * 512],
                        in_=osb)
    nc.compile()
    return nc


def _get_compiled():
    global _compiled
    if _compiled is None:
        _compiled = _build()
    return _compiled


def _fix_ties(x_flat):
    g = np.abs(x_flat.reshape(-1, 4))
    m1 = np.maximum(g[:, 0], g[:, 1]); n1 = np.minimum(g[:, 0], g[:, 1])
    m2 = np.maximum(g[:, 2], g[:, 3]); n2 = np.minimum(g[:, 2], g[:, 3])
    thr = np.maximum(np.minimum(m1, m2), np.maximum(n1, n2))
    third = np.minimum(np.minimum(m1, m2), np.maximum(n1, n2))
    tied = np.flatnonzero(thr == third)
    if len(tied) == 0:
        return x_flat
    x_flat = x_flat.copy()
    gv = x_flat.reshape(-1, 4)
    for t in tied:
        row = gv[t]
        order = np.argsort(-np.abs(row), kind="stable")
        row[order[2:]] = 0.0
    return x_flat


def _quant_weights(weight):
    import ml_dtypes
    wT = np.ascontiguousarray(weight.T, dtype=np.float32)
    return wT.astype(ml_dtypes.bfloat16)


def _prep_x(x_flat):
    return _fix_ties(np.ascontiguousarray(x_flat, dtype=np.float32))


def kernel(x: np.ndarray, weight: np.ndarray) -> np.ndarray:
    from concourse.bass_utils import run_bass_kernel_spmd

    nc = _get_compiled()
    x_flat = _prep_x(x.reshape(TOK_TOTAL, D))
    wb = _quant_weights(weight)
    in_maps = [{"xs": x_flat[c * TOK:(c + 1) * TOK], "wb": wb}
               for c in range(N_CORES)]
    res = run_bass_kernel_spmd(nc, in_maps, core_ids=list(range(N_CORES)))
    out = np.concatenate([res.results[c]["o"] for c in range(N_CORES)], axis=0)
    return out.reshape(BS, SEQ, OUTF)
